# revision 25
# baseline (speedup 1.0000x reference)
"""DMRG two-site effective Hamiltonian application (ApplyMPO) on 8 trn2 cores.

Math (reference):
  res[h,i,j,k] = sum_{a,b,c,d,e,f,g} L[b,h,a] M1[b,d,i,c] M2[d,f,j,e]
                                     R[f,k,g] psi[a,c,e,g]

Device algorithm (per core, output bond h sharded 8 x 128), all fp16:
  Q[(c,e,b),(i,j,f)] = sum_d M1[b,d,i,c] M2[d,f,j,e]            (host, 400 els)
  step1: T1g[g; h,ce,b] = sum_a psi[a,ce,g] L[b,h,a]   (PE, K=a, g-major out)
  flipA: T1P[(h6,ce,b); g] = transpose of T1g 6-h-column packs  (PE transpose)
  mix:   T3G[g; (ijf,h6)] = T1P^T @ Q6P  (Q6P = block-diag Q over h6)
  step4: res[h; (i,j),k] += T3G[:,ijf,:]^T @ R^T[f][g,k]        (PE, K=g, acc f)
Single transpose stage (mix lands g-major for step4); PE stream is software
pipelined: flip/mix of gblk-1 is emitted after step1 of gblk, so evacuation
latency never blocks ready matmuls.
"""

import numpy as np

import concourse.bacc as bacc
import concourse.mybir as mybir
import concourse.tile as tile
from concourse import bass_utils

F32 = mybir.dt.float32
FP16 = mybir.dt.float16

CHI = 1024
W = 5
D = 2
NCORES = 8
H = CHI // NCORES  # 128, h rows per core

_nc_cache = None


def _build_nc():
    nc = bacc.Bacc("TRN2", target_bir_lowering=False)
    # host-prearranged:
    #   psi[gblk, a_lo, ce, ac, g_lo]  (lhsT tiles for step1: [a; g] per (ce,ac))
    #   lt[a_lo, ac, h, b]             (rhs for step1: [a; (h,b)] per ac)
    #   rt[gblk, g_lo, f, k]           (rhs for step4)
    psi = nc.dram_tensor("psi", [8, 128, 4, 8, 128], FP16, kind="ExternalInput")
    lt = nc.dram_tensor("lt", [128, 8, H, 5], FP16, kind="ExternalInput")
    rt = nc.dram_tensor("rt", [8, 128, 5, 1024], FP16, kind="ExternalInput")
    q6 = nc.dram_tensor("q6", [120, 128], FP16, kind="ExternalInput")
    q2 = nc.dram_tensor("q2", [40, 128], FP16, kind="ExternalInput")
    idn = nc.dram_tensor("idn", [128, 128], FP16, kind="ExternalInput")
    res = nc.dram_tensor("res", [H, 4096], F32, kind="ExternalOutput")  # h;(i,j,k)

    with tile.TileContext(nc) as tc:
        with (
            tc.tile_pool(name="const", bufs=1) as const_pool,
            tc.tile_pool(name="psis", bufs=3) as psi_pool,
            tc.tile_pool(name="t1", bufs=3) as t1_pool,
            tc.tile_pool(name="t1p", bufs=3) as t1p_pool,
            tc.tile_pool(name="t3g", bufs=5) as t3g_pool,
            tc.tile_pool(name="rblk", bufs=5) as rblk_pool,
            tc.tile_pool(name="resp", bufs=1) as res_pool,
            tc.tile_pool(name="ps_s1a", bufs=2, space="PSUM") as ps_s1a,
            tc.tile_pool(name="ps_s1b", bufs=1, space="PSUM") as ps_s1b,
            tc.tile_pool(name="ps_mid", bufs=3, space="PSUM") as ps_mid,
            tc.tile_pool(name="ps_s4", bufs=2, space="PSUM") as ps_s4,
        ):
            # ---- static loads ----
            lt_sb = const_pool.tile([128, 8, H * 5], FP16)  # [a_lo; ac, (h,b)]
            lt_r = lt.ap().rearrange("p ac h b -> p ac (h b)")
            nc.gpsimd.dma_start(lt_sb[:, 0], lt_r[:, 0])
            q6_sb = const_pool.tile([120, 128], FP16)
            q2_sb = const_pool.tile([40, 128], FP16)
            idn_sb = const_pool.tile([128, 128], FP16)
            res_sb = res_pool.tile([128, 4096], F32)

            def load_rest_of_consts():
                # parallelize startup DMA issue across idle engine queues
                # (only gpsimd/SP/ACT may initiate DMAs)
                for ac in range(1, 8):
                    nc.gpsimd.dma_start(lt_sb[:, ac], lt_r[:, ac])
                nc.scalar.dma_start(q6_sb[:], q6.ap())
                nc.scalar.dma_start(q2_sb[:], q2.ap())
                nc.scalar.dma_start(idn_sb[:], idn.ap())

            evac_ct = 0

            def evac_copy(out, in_):
                # only DVE and ACT can read PSUM; alternate them 1:1
                nonlocal evac_ct
                evac_ct += 1
                if evac_ct % 2 == 0:
                    nc.scalar.copy(out, in_)
                else:
                    nc.vector.tensor_copy(out, in_)

            pending_s4_emitters = []
            t3gs = []
            rblks = []

            def step1(gblk):
                psig = psi_pool.tile([128, 4, 8, 128], FP16, tag="psi")
                if gblk == 0:
                    # per-ce slices on the gpsimd queue: first chain starts early
                    nc.gpsimd.dma_start(psig[:, 0], psi.ap()[gblk, :, 0])
                    load_rest_of_consts()
                    for ce in range(1, 4):
                        nc.gpsimd.dma_start(psig[:, ce], psi.ap()[gblk, :, ce])
                else:
                    nc.sync.dma_start(psig[:], psi.ap()[gblk])
                t1g = t1_pool.tile([128, 128, 4, 5], FP16, tag="t1g")  # g;h,ce,b
                for ce in range(4):
                    p1a = ps_s1a.tile([128, 510], F32, tag="s1a")
                    p1b = ps_s1b.tile([128, 130], F32, tag="s1b")
                    for ac in range(8):
                        lhsT = psig[:, ce, ac]  # [a; g]
                        nc.tensor.matmul(
                            p1a[:],
                            lhsT,
                            lt_sb[:, ac, 0:510],
                            start=(ac == 0),
                            stop=(ac == 7),
                        )
                        nc.tensor.matmul(
                            p1b[:],
                            lhsT,
                            lt_sb[:, ac, 510:640],
                            start=(ac == 0),
                            stop=(ac == 7),
                        )
                    # psum [g; (h,b)] -> t1g[g; h, ce, b]: runs of 5 both sides
                    evac_copy(
                        t1g[:, 0:102, ce, :],
                        p1a[:].rearrange("p (h b) -> p h b", b=5),
                    )
                    evac_copy(
                        t1g[:, 102:128, ce, :],
                        p1b[:].rearrange("p (h b) -> p h b", b=5),
                    )
                    if pending_s4_emitters:
                        pending_s4_emitters[0][0](ce)
                        if ce == 3:
                            pending_s4_emitters.pop(0)
                return t1g

            def flip_mix(t1g):
                # T3G[g; ijf, h] from T1g, one transpose stage + block-diag Q
                t3g = t3g_pool.tile([128, 20, 128], FP16, tag="t3g")
                t1g_flat = t1g[:].rearrange("p h ce b -> p (h ce b)")
                # 5 groups of 4 six-packs (h 0..119), then (6,2) tail (h 120..127)
                for grp in range(5):
                    pa32 = ps_mid.tile([128, 512], F32, tag="mid")
                    pa = pa32[:].bitcast(FP16)  # [128, 1024] fp16 view
                    pm = ps_mid.tile([128, 512], F32, tag="mid")
                    t1p = t1p_pool.tile([120, 512], FP16, tag="t1p")
                    for pp in range(4):
                        base = (grp * 4 + pp) * 120
                        nc.tensor.transpose(
                            pa[0:120, pp * 128:pp * 128 + 128],
                            t1g_flat[:, base:base + 120],
                            idn_sb[:],
                        )
                    evac_copy(t1p[:], pa[0:120, 0:512])
                    for pp in range(4):
                        nc.tensor.matmul(
                            pm[:, pp * 128:pp * 128 + 120],
                            t1p[:, pp * 128:(pp + 1) * 128],
                            q6_sb[:, 0:120],
                            start=True,
                            stop=True,
                        )
                    # pm [g; (pp, ijf, h6)] -> t3g[g; ijf, 24h-slice]
                    evac_copy(
                        t3g[:, :, grp * 24:(grp + 1) * 24].rearrange(
                            "p i (pp h) -> p pp i h", pp=4
                        ),
                        pm[:].rearrange("p (pp x) -> p pp x", pp=4)[:, :, 0:120]
                        .rearrange("p pp (i h) -> p pp i h", h=6),
                    )
                # tail: one 6-pack (h 120..125) + one 2-pack (h 126..127)
                pa32 = ps_mid.tile([128, 512], F32, tag="mid")
                pa = pa32[:].bitcast(FP16)
                pm = ps_mid.tile([128, 512], F32, tag="mid")
                t1p = t1p_pool.tile([120, 512], FP16, tag="t1p")
                nc.tensor.transpose(
                    pa[0:120, 0:128], t1g_flat[:, 2400:2520], idn_sb[:]
                )
                nc.tensor.transpose(
                    pa[0:40, 128:256], t1g_flat[:, 2520:2560], idn_sb[:]
                )
                evac_copy(t1p[:, 0:256], pa[0:120, 0:256])
                nc.tensor.matmul(
                    pm[:, 0:120], t1p[:, 0:128], q6_sb[:, 0:120],
                    start=True, stop=True,
                )
                nc.tensor.matmul(
                    pm[:, 128:168], t1p[0:40, 128:256], q2_sb[:, 0:40],
                    start=True, stop=True,
                )
                evac_copy(
                    t3g[:, :, 120:126],
                    pm[:, 0:120].rearrange("p (i h) -> p i h", h=6),
                )
                evac_copy(
                    t3g[:, :, 126:128],
                    pm[:, 128:168].rearrange("p (i h) -> p i h", h=2),
                )
                t3gs.append(t3g)

            def load_rblk(gblk):
                rblk = rblk_pool.tile([128, 5, 1024], FP16, tag="rblk")
                nc.sync.dma_start(rblk[:], rt.ap()[gblk])
                rblks.append(rblk)

            def make_s4(qq, t3gs_, rblks_):
                def emit_ij(ij):
                    for kh in range(2):
                        ps4 = ps_s4.tile([128, 512], F32, tag="s4")  # 1 bank
                        for half in range(2):
                            for f in range(5):
                                lhsT = t3gs_[half][:, ij * 5 + f, :]
                                nc.tensor.matmul(
                                    ps4[:],
                                    lhsT,
                                    rblks_[half][:, f, kh * 512:(kh + 1) * 512],
                                    start=(half == 0 and f == 0),
                                    stop=(half == 1 and f == 4),
                                )
                        dst = res_sb[:, ij * 1024 + kh * 512:ij * 1024 + (kh + 1) * 512]
                        if qq == 0:
                            evac_copy(dst, ps4[:])
                        else:
                            nc.vector.tensor_add(dst, dst, ps4[:])
                    if qq == 3:
                        nc.sync.dma_start(
                            res.ap()[:, ij * 1024:(ij + 1) * 1024],
                            res_sb[:, ij * 1024:(ij + 1) * 1024],
                        )

                return emit_ij

            # software pipeline: flip/mix trails step1 by one gblk
            prev_t1g = None
            for gblk in range(8):
                t1g = step1(gblk)
                load_rblk(gblk)
                if prev_t1g is not None:
                    flip_mix(prev_t1g)
                    if gblk % 2 == 0:  # finished t3g of gblk-1 (odd): pair done
                        q = (gblk - 1) // 2
                        pending_s4_emitters.append(
                            (make_s4(q, t3gs[-2:], rblks[gblk - 2:gblk]), 4)
                        )
                prev_t1g = t1g
            flip_mix(prev_t1g)  # gblk 7
            pending_s4_emitters.append((make_s4(3, t3gs[-2:], rblks[6:8]), 4))

            # flush remaining deferred step-4 work
            for emit, n in pending_s4_emitters:
                for ij in range(n):
                    emit(ij)
            pending_s4_emitters.clear()
    nc.compile()
    return nc


def _host_inputs(psi_flat, L, M1, M2, R):
    # psi[a,ce,g]: a=ac*128+a_lo, g=gblk*128+g_lo -> [gblk, a_lo, ce, ac, g_lo]
    psi = np.ascontiguousarray(
        psi_flat.reshape(8, 128, 4, 8, 128).transpose(3, 1, 2, 0, 4)
    ).astype(np.float16)
    # R[f,k,g] -> [gblk, g_lo, f, k]
    RT = np.ascontiguousarray(
        R.transpose(2, 0, 1).reshape(8, 128, 5, 1024)
    ).astype(np.float16)
    # Q rows permuted to (ce, b) order to match t1g layout [h, ce, b]
    Q = np.einsum("bdic,dfje->cebijf", M1, M2).reshape(20, 20).astype(np.float32)
    rows = np.arange(20)
    Q6P = np.zeros((120, 128), np.float32)
    for g6 in range(6):
        Q6P[np.ix_(g6 * 20 + rows, rows * 6 + g6)] = Q
    Q2P = np.zeros((40, 128), np.float32)
    for g2 in range(2):
        Q2P[np.ix_(g2 * 20 + rows, rows * 2 + g2)] = Q
    idn = np.eye(128, dtype=np.float16)
    q6h = Q6P.astype(np.float16)
    q2h = Q2P.astype(np.float16)
    in_maps = []
    for c in range(NCORES):
        # L[b, h_shard, a] -> lt[a_lo, ac, h, b]
        LT = np.ascontiguousarray(
            L[:, c * H:(c + 1) * H, :].reshape(5, H, 8, 128).transpose(3, 2, 1, 0)
        ).astype(np.float16)
        in_maps.append({"psi": psi, "lt": LT, "rt": RT, "q6": q6h, "q2": q2h, "idn": idn})
    return in_maps


def kernel(**inputs):
    psi_flat = np.asarray(inputs["psi_flat"], np.float32)
    L = np.asarray(inputs["L"], np.float32)
    M1 = np.asarray(inputs["M1"], np.float32)
    M2 = np.asarray(inputs["M2"], np.float32)
    R = np.asarray(inputs["R"], np.float32)

    global _nc_cache
    if _nc_cache is None:
        _nc_cache = _build_nc()
    nc = _nc_cache

    in_maps = _host_inputs(psi_flat, L, M1, M2, R)
    out = bass_utils.run_bass_kernel_spmd(nc, in_maps, core_ids=list(range(NCORES)))
    parts = [out.results[c]["res"] for c in range(NCORES)]
    return np.concatenate(parts, axis=0).reshape(-1)


# revision 27
# speedup vs baseline: 1.0582x; 1.0582x over previous
"""DMRG two-site effective Hamiltonian application (ApplyMPO) on 8 trn2 cores.

Math (reference):
  res[h,i,j,k] = sum_{a,b,c,d,e,f,g} L[b,h,a] M1[b,d,i,c] M2[d,f,j,e]
                                     R[f,k,g] psi[a,c,e,g]

Device algorithm (per core, output bond h sharded 8 x 128), all fp16:
  Q[(c,e,b),(i,j,f)] = sum_d M1[b,d,i,c] M2[d,f,j,e]            (host, 400 els)
  step1: T1g[g; h,ce,b] = sum_a psi[a,ce,g] L[b,h,a]   (PE, K=a, g-major out)
  flipA: T1P[(h6,ce,b); g] = transpose of T1g 6-h-column packs  (PE transpose)
  mix:   T3G[g; (ijf,h6)] = T1P^T @ Q6P  (Q6P = block-diag Q over h6)
  step4: res[h; (i,j),k] += T3G[:,ijf,:]^T @ R^T[f][g,k]        (PE, K=g, acc f)
Single transpose stage (mix lands g-major for step4); PE stream is software
pipelined: flip/mix of gblk-1 is emitted after step1 of gblk, so evacuation
latency never blocks ready matmuls.
"""

import numpy as np

import concourse.bacc as bacc
import concourse.mybir as mybir
import concourse.tile as tile
from concourse import bass_utils

F32 = mybir.dt.float32
FP16 = mybir.dt.float16

CHI = 1024
W = 5
D = 2
NCORES = 8
H = CHI // NCORES  # 128, h rows per core

_nc_cache = None


def _build_nc():
    nc = bacc.Bacc("TRN2", target_bir_lowering=False)
    # host-prearranged:
    #   psi[gblk, a_lo, ce, ac, g_lo]  (lhsT tiles for step1: [a; g] per (ce,ac))
    #   lt[a_lo, ac, h, b]             (rhs for step1: [a; (h,b)] per ac)
    #   rt[gblk, g_lo, f, k]           (rhs for step4)
    psi = nc.dram_tensor("psi", [8, 128, 4, 8, 128], FP16, kind="ExternalInput")
    lt = nc.dram_tensor("lt", [128, 8, H, 5], FP16, kind="ExternalInput")
    rt = nc.dram_tensor("rt", [8, 128, 5, 1024], FP16, kind="ExternalInput")
    q6 = nc.dram_tensor("q6", [120, 128], FP16, kind="ExternalInput")
    q2 = nc.dram_tensor("q2", [40, 128], FP16, kind="ExternalInput")
    idn = nc.dram_tensor("idn", [128, 128], FP16, kind="ExternalInput")
    res = nc.dram_tensor("res", [H, 4096], F32, kind="ExternalOutput")  # h;(i,j,k)

    with tile.TileContext(nc) as tc:
        with (
            tc.tile_pool(name="const", bufs=1) as const_pool,
            tc.tile_pool(name="psis", bufs=3) as psi_pool,
            tc.tile_pool(name="t1", bufs=3) as t1_pool,
            tc.tile_pool(name="t1p", bufs=3) as t1p_pool,
            tc.tile_pool(name="t3g", bufs=5) as t3g_pool,
            tc.tile_pool(name="rblk", bufs=5) as rblk_pool,
            tc.tile_pool(name="resp", bufs=1) as res_pool,
            tc.tile_pool(name="ps_s1a", bufs=2, space="PSUM") as ps_s1a,
            tc.tile_pool(name="ps_s1b", bufs=1, space="PSUM") as ps_s1b,
            tc.tile_pool(name="ps_mid", bufs=3, space="PSUM") as ps_mid,
            tc.tile_pool(name="ps_s4", bufs=2, space="PSUM") as ps_s4,
        ):
            # ---- static loads ----
            lt_sb = const_pool.tile([128, 8, H * 5], FP16)  # [a_lo; ac, (h,b)]
            lt_r = lt.ap().rearrange("p ac h b -> p ac (h b)")
            nc.sync.dma_start(lt_sb[:, 0], lt_r[:, 0])
            q6_sb = const_pool.tile([120, 128], FP16)
            q2_sb = const_pool.tile([40, 128], FP16)
            idn_sb = const_pool.tile([128, 128], FP16)
            res_sb = res_pool.tile([128, 4096], F32)

            evac_ct = 0

            def evac_copy(out, in_):
                # only DVE and ACT can read PSUM; alternate them 1:1
                nonlocal evac_ct
                evac_ct += 1
                if evac_ct % 2 == 0:
                    nc.scalar.copy(out, in_)
                else:
                    nc.vector.tensor_copy(out, in_)

            pending_s4_emitters = []
            t3gs = []
            rblks = []

            def step1(gblk):
                psig = psi_pool.tile([128, 4, 8, 128], FP16, tag="psi")
                if gblk == 0:
                    # per-ce psi slices interleaved with lt chunks so the first
                    # step1 chain starts as early as possible
                    nc.sync.dma_start(psig[:, 0], psi.ap()[gblk, :, 0])
                    for ac in range(1, 4):
                        nc.sync.dma_start(lt_sb[:, ac], lt_r[:, ac])
                    nc.sync.dma_start(psig[:, 1], psi.ap()[gblk, :, 1])
                    for ac in range(4, 8):
                        nc.sync.dma_start(lt_sb[:, ac], lt_r[:, ac])
                    nc.sync.dma_start(psig[:, 2], psi.ap()[gblk, :, 2])
                    nc.sync.dma_start(q6_sb[:], q6.ap())
                    nc.sync.dma_start(psig[:, 3], psi.ap()[gblk, :, 3])
                    nc.sync.dma_start(q2_sb[:], q2.ap())
                    nc.sync.dma_start(idn_sb[:], idn.ap())
                else:
                    nc.sync.dma_start(psig[:], psi.ap()[gblk])
                t1g = t1_pool.tile([128, 128, 4, 5], FP16, tag="t1g")  # g;h,ce,b
                for ce in range(4):
                    p1a = ps_s1a.tile([128, 510], F32, tag="s1a")
                    p1b = ps_s1b.tile([128, 130], F32, tag="s1b")
                    for ac in range(8):
                        lhsT = psig[:, ce, ac]  # [a; g]
                        nc.tensor.matmul(
                            p1a[:],
                            lhsT,
                            lt_sb[:, ac, 0:510],
                            start=(ac == 0),
                            stop=(ac == 7),
                        )
                        nc.tensor.matmul(
                            p1b[:],
                            lhsT,
                            lt_sb[:, ac, 510:640],
                            start=(ac == 0),
                            stop=(ac == 7),
                        )
                    # psum [g; (h,b)] -> t1g[g; h, ce, b]: runs of 5 both sides
                    evac_copy(
                        t1g[:, 0:102, ce, :],
                        p1a[:].rearrange("p (h b) -> p h b", b=5),
                    )
                    evac_copy(
                        t1g[:, 102:128, ce, :],
                        p1b[:].rearrange("p (h b) -> p h b", b=5),
                    )
                    if pending_s4_emitters:
                        pending_s4_emitters[0][0](ce)
                        if ce == 3:
                            pending_s4_emitters.pop(0)
                return t1g

            def flip_mix(t1g):
                # T3G[g; ijf, h] from T1g, one transpose stage + block-diag Q
                t3g = t3g_pool.tile([128, 20, 128], FP16, tag="t3g")
                t1g_flat = t1g[:].rearrange("p h ce b -> p (h ce b)")
                # 5 groups of 4 six-packs (h 0..119), then (6,2) tail (h 120..127)
                for grp in range(5):
                    pa32 = ps_mid.tile([128, 512], F32, tag="mid")
                    pa = pa32[:].bitcast(FP16)  # [128, 1024] fp16 view
                    pm = ps_mid.tile([128, 512], F32, tag="mid")
                    t1p = t1p_pool.tile([120, 512], FP16, tag="t1p")
                    for pp in range(4):
                        base = (grp * 4 + pp) * 120
                        nc.tensor.transpose(
                            pa[0:120, pp * 128:pp * 128 + 128],
                            t1g_flat[:, base:base + 120],
                            idn_sb[:],
                        )
                    evac_copy(t1p[:], pa[0:120, 0:512])
                    for pp in range(4):
                        nc.tensor.matmul(
                            pm[:, pp * 128:pp * 128 + 120],
                            t1p[:, pp * 128:(pp + 1) * 128],
                            q6_sb[:, 0:120],
                            start=True,
                            stop=True,
                        )
                    # pm [g; (pp, ijf, h6)] -> t3g[g; ijf, 24h-slice]
                    evac_copy(
                        t3g[:, :, grp * 24:(grp + 1) * 24].rearrange(
                            "p i (pp h) -> p pp i h", pp=4
                        ),
                        pm[:].rearrange("p (pp x) -> p pp x", pp=4)[:, :, 0:120]
                        .rearrange("p pp (i h) -> p pp i h", h=6),
                    )
                # tail: one 6-pack (h 120..125) + one 2-pack (h 126..127)
                pa32 = ps_mid.tile([128, 512], F32, tag="mid")
                pa = pa32[:].bitcast(FP16)
                pm = ps_mid.tile([128, 512], F32, tag="mid")
                t1p = t1p_pool.tile([120, 512], FP16, tag="t1p")
                nc.tensor.transpose(
                    pa[0:120, 0:128], t1g_flat[:, 2400:2520], idn_sb[:]
                )
                nc.tensor.transpose(
                    pa[0:40, 128:256], t1g_flat[:, 2520:2560], idn_sb[:]
                )
                evac_copy(t1p[:, 0:256], pa[0:120, 0:256])
                nc.tensor.matmul(
                    pm[:, 0:120], t1p[:, 0:128], q6_sb[:, 0:120],
                    start=True, stop=True,
                )
                nc.tensor.matmul(
                    pm[:, 128:168], t1p[0:40, 128:256], q2_sb[:, 0:40],
                    start=True, stop=True,
                )
                evac_copy(
                    t3g[:, :, 120:126],
                    pm[:, 0:120].rearrange("p (i h) -> p i h", h=6),
                )
                evac_copy(
                    t3g[:, :, 126:128],
                    pm[:, 128:168].rearrange("p (i h) -> p i h", h=2),
                )
                t3gs.append(t3g)

            def load_rblk(gblk):
                rblk = rblk_pool.tile([128, 5, 1024], FP16, tag="rblk")
                nc.sync.dma_start(rblk[:], rt.ap()[gblk])
                rblks.append(rblk)

            def make_s4(qq, t3gs_, rblks_):
                def emit_ij(ij):
                    for kh in range(2):
                        ps4 = ps_s4.tile([128, 512], F32, tag="s4")  # 1 bank
                        for half in range(2):
                            for f in range(5):
                                lhsT = t3gs_[half][:, ij * 5 + f, :]
                                nc.tensor.matmul(
                                    ps4[:],
                                    lhsT,
                                    rblks_[half][:, f, kh * 512:(kh + 1) * 512],
                                    start=(half == 0 and f == 0),
                                    stop=(half == 1 and f == 4),
                                )
                        dst = res_sb[:, ij * 1024 + kh * 512:ij * 1024 + (kh + 1) * 512]
                        if qq == 0:
                            evac_copy(dst, ps4[:])
                        else:
                            nc.vector.tensor_add(dst, dst, ps4[:])
                    if qq == 3:
                        nc.sync.dma_start(
                            res.ap()[:, ij * 1024:(ij + 1) * 1024],
                            res_sb[:, ij * 1024:(ij + 1) * 1024],
                        )

                return emit_ij

            # software pipeline: flip/mix trails step1 by one gblk
            prev_t1g = None
            for gblk in range(8):
                t1g = step1(gblk)
                load_rblk(gblk)
                if prev_t1g is not None:
                    flip_mix(prev_t1g)
                    if gblk % 2 == 0:  # finished t3g of gblk-1 (odd): pair done
                        q = (gblk - 1) // 2
                        pending_s4_emitters.append(
                            (make_s4(q, t3gs[-2:], rblks[gblk - 2:gblk]), 4)
                        )
                prev_t1g = t1g
            flip_mix(prev_t1g)  # gblk 7
            pending_s4_emitters.append((make_s4(3, t3gs[-2:], rblks[6:8]), 4))

            # flush remaining deferred step-4 work
            for emit, n in pending_s4_emitters:
                for ij in range(n):
                    emit(ij)
            pending_s4_emitters.clear()
    nc.compile()
    return nc


def _host_inputs(psi_flat, L, M1, M2, R):
    # psi[a,ce,g]: a=ac*128+a_lo, g=gblk*128+g_lo -> [gblk, a_lo, ce, ac, g_lo]
    psi = np.ascontiguousarray(
        psi_flat.reshape(8, 128, 4, 8, 128).transpose(3, 1, 2, 0, 4)
    ).astype(np.float16)
    # R[f,k,g] -> [gblk, g_lo, f, k]
    RT = np.ascontiguousarray(
        R.transpose(2, 0, 1).reshape(8, 128, 5, 1024)
    ).astype(np.float16)
    # Q rows permuted to (ce, b) order to match t1g layout [h, ce, b]
    Q = np.einsum("bdic,dfje->cebijf", M1, M2).reshape(20, 20).astype(np.float32)
    rows = np.arange(20)
    Q6P = np.zeros((120, 128), np.float32)
    for g6 in range(6):
        Q6P[np.ix_(g6 * 20 + rows, rows * 6 + g6)] = Q
    Q2P = np.zeros((40, 128), np.float32)
    for g2 in range(2):
        Q2P[np.ix_(g2 * 20 + rows, rows * 2 + g2)] = Q
    idn = np.eye(128, dtype=np.float16)
    q6h = Q6P.astype(np.float16)
    q2h = Q2P.astype(np.float16)
    in_maps = []
    for c in range(NCORES):
        # L[b, h_shard, a] -> lt[a_lo, ac, h, b]
        LT = np.ascontiguousarray(
            L[:, c * H:(c + 1) * H, :].reshape(5, H, 8, 128).transpose(3, 2, 1, 0)
        ).astype(np.float16)
        in_maps.append({"psi": psi, "lt": LT, "rt": RT, "q6": q6h, "q2": q2h, "idn": idn})
    return in_maps


def kernel(**inputs):
    psi_flat = np.asarray(inputs["psi_flat"], np.float32)
    L = np.asarray(inputs["L"], np.float32)
    M1 = np.asarray(inputs["M1"], np.float32)
    M2 = np.asarray(inputs["M2"], np.float32)
    R = np.asarray(inputs["R"], np.float32)

    global _nc_cache
    if _nc_cache is None:
        _nc_cache = _build_nc()
    nc = _nc_cache

    in_maps = _host_inputs(psi_flat, L, M1, M2, R)
    out = bass_utils.run_bass_kernel_spmd(nc, in_maps, core_ids=list(range(NCORES)))
    parts = [out.results[c]["res"] for c in range(NCORES)]
    return np.concatenate(parts, axis=0).reshape(-1)


# revision 28
# speedup vs baseline: 1.0586x; 1.0004x over previous
"""DMRG two-site effective Hamiltonian application (ApplyMPO) on 8 trn2 cores.

Math (reference):
  res[h,i,j,k] = sum_{a,b,c,d,e,f,g} L[b,h,a] M1[b,d,i,c] M2[d,f,j,e]
                                     R[f,k,g] psi[a,c,e,g]

Device algorithm (per core, output bond h sharded 8 x 128), all fp16:
  Q[(c,e,b),(i,j,f)] = sum_d M1[b,d,i,c] M2[d,f,j,e]            (host, 400 els)
  step1: T1g[g; h,ce,b] = sum_a psi[a,ce,g] L[b,h,a]   (PE, K=a, g-major out)
  flipA: T1P[(h6,ce,b); g] = transpose of T1g 6-h-column packs  (PE transpose)
  mix:   T3G[g; (ijf,h6)] = T1P^T @ Q6P  (Q6P = block-diag Q over h6)
  step4: res[h; (i,j),k] += T3G[:,ijf,:]^T @ R^T[f][g,k]        (PE, K=g, acc f)
Single transpose stage (mix lands g-major for step4); PE stream is software
pipelined: flip/mix of gblk-1 is emitted after step1 of gblk, so evacuation
latency never blocks ready matmuls.
"""

import numpy as np

import concourse.bacc as bacc
import concourse.mybir as mybir
import concourse.tile as tile
from concourse import bass_utils

F32 = mybir.dt.float32
FP16 = mybir.dt.float16

CHI = 1024
W = 5
D = 2
NCORES = 8
H = CHI // NCORES  # 128, h rows per core

_nc_cache = None


def _build_nc():
    nc = bacc.Bacc("TRN2", target_bir_lowering=False)
    # host-prearranged:
    #   psi[gblk, a_lo, ce, ac, g_lo]  (lhsT tiles for step1: [a; g] per (ce,ac))
    #   lt[a_lo, ac, h, b]             (rhs for step1: [a; (h,b)] per ac)
    #   rt[gblk, g_lo, f, k]           (rhs for step4)
    psi = nc.dram_tensor("psi", [8, 128, 4, 8, 128], FP16, kind="ExternalInput")
    lt = nc.dram_tensor("lt", [128, 8, H, 5], FP16, kind="ExternalInput")
    rt = nc.dram_tensor("rt", [8, 128, 5, 1024], FP16, kind="ExternalInput")
    q6 = nc.dram_tensor("q6", [120, 128], FP16, kind="ExternalInput")
    q2 = nc.dram_tensor("q2", [40, 128], FP16, kind="ExternalInput")
    idn = nc.dram_tensor("idn", [128, 128], FP16, kind="ExternalInput")
    res = nc.dram_tensor("res", [H, 4096], F32, kind="ExternalOutput")  # h;(i,j,k)

    with tile.TileContext(nc) as tc:
        with (
            tc.tile_pool(name="const", bufs=1) as const_pool,
            tc.tile_pool(name="psis", bufs=3) as psi_pool,
            tc.tile_pool(name="t1", bufs=3) as t1_pool,
            tc.tile_pool(name="t1p", bufs=3) as t1p_pool,
            tc.tile_pool(name="t3g", bufs=5) as t3g_pool,
            tc.tile_pool(name="rblk", bufs=5) as rblk_pool,
            tc.tile_pool(name="resp", bufs=1) as res_pool,
            tc.tile_pool(name="ps_s1a", bufs=2, space="PSUM") as ps_s1a,
            tc.tile_pool(name="ps_mid", bufs=3, space="PSUM") as ps_mid,
            tc.tile_pool(name="ps_s4", bufs=3, space="PSUM") as ps_s4,
        ):
            # ---- static loads ----
            lt_sb = const_pool.tile([128, 8, H * 5], FP16)  # [a_lo; ac, (h,b)]
            lt_r = lt.ap().rearrange("p ac h b -> p ac (h b)")
            nc.sync.dma_start(lt_sb[:, 0], lt_r[:, 0])
            q6_sb = const_pool.tile([120, 128], FP16)
            q2_sb = const_pool.tile([40, 128], FP16)
            idn_sb = const_pool.tile([128, 128], FP16)
            res_sb = res_pool.tile([128, 4096], F32)

            evac_ct = 0
            evac_scalar_bias = [False]

            def evac_copy(out, in_):
                # only DVE and ACT can read PSUM; alternate them 1:1.
                # Under scalar bias (endgame), give ACT 2 of 3 so the DVE
                # queue stays clear for the step-4 res adds.
                nonlocal evac_ct
                evac_ct += 1
                use_scalar = (
                    evac_ct % 3 != 0 if evac_scalar_bias[0] else evac_ct % 2 == 0
                )
                if use_scalar:
                    nc.scalar.copy(out, in_)
                else:
                    nc.vector.tensor_copy(out, in_)

            pending_s4_emitters = []
            t3gs = []
            rblks = []

            def step1(gblk):
                psig = psi_pool.tile([128, 4, 8, 128], FP16, tag="psi")
                if gblk == 0:
                    # per-ce psi slices interleaved with lt chunks so the first
                    # step1 chain starts as early as possible
                    nc.sync.dma_start(psig[:, 0, 0:4], psi.ap()[gblk, :, 0, 0:4])
                    nc.sync.dma_start(psig[:, 0, 4:8], psi.ap()[gblk, :, 0, 4:8])
                    for ac in range(1, 4):
                        nc.sync.dma_start(lt_sb[:, ac], lt_r[:, ac])
                    nc.sync.dma_start(psig[:, 1], psi.ap()[gblk, :, 1])
                    for ac in range(4, 8):
                        nc.sync.dma_start(lt_sb[:, ac], lt_r[:, ac])
                    nc.sync.dma_start(psig[:, 2], psi.ap()[gblk, :, 2])
                    nc.sync.dma_start(q6_sb[:], q6.ap())
                    nc.sync.dma_start(psig[:, 3], psi.ap()[gblk, :, 3])
                    nc.sync.dma_start(q2_sb[:], q2.ap())
                    nc.sync.dma_start(idn_sb[:], idn.ap())
                else:
                    nc.sync.dma_start(psig[:], psi.ap()[gblk])
                t1g = t1_pool.tile([128, 128, 4, 5], FP16, tag="t1g")  # g;h,ce,b
                for ce in range(4):
                    p1a = ps_s1a.tile([128, 510], F32, tag="s1a")
                    p1b32 = ps_mid.tile([128, 512], F32, tag="mid")
                    p1b = p1b32[:, 0:130]
                    for ac in range(8):
                        lhsT = psig[:, ce, ac]  # [a; g]
                        nc.tensor.matmul(
                            p1a[:],
                            lhsT,
                            lt_sb[:, ac, 0:510],
                            start=(ac == 0),
                            stop=(ac == 7),
                        )
                        nc.tensor.matmul(
                            p1b,
                            lhsT,
                            lt_sb[:, ac, 510:640],
                            start=(ac == 0),
                            stop=(ac == 7),
                        )
                    # psum [g; (h,b)] -> t1g[g; h, ce, b]: runs of 5 both sides
                    evac_copy(
                        t1g[:, 0:102, ce, :],
                        p1a[:].rearrange("p (h b) -> p h b", b=5),
                    )
                    evac_copy(
                        t1g[:, 102:128, ce, :],
                        p1b.rearrange("p (h b) -> p h b", b=5),
                    )
                    if pending_s4_emitters:
                        pending_s4_emitters[0][0](ce)
                        if ce == 3:
                            pending_s4_emitters.pop(0)
                return t1g

            def flip_mix(t1g):
                # T3G[g; ijf, h] from T1g, one transpose stage + block-diag Q
                t3g = t3g_pool.tile([128, 20, 128], FP16, tag="t3g")
                t1g_flat = t1g[:].rearrange("p h ce b -> p (h ce b)")
                # 5 groups of 4 six-packs (h 0..119), then (6,2) tail (h 120..127)
                for grp in range(5):
                    pa32 = ps_mid.tile([128, 512], F32, tag="mid")
                    pa = pa32[:].bitcast(FP16)  # [128, 1024] fp16 view
                    pm = ps_mid.tile([128, 512], F32, tag="mid")
                    t1p = t1p_pool.tile([120, 512], FP16, tag="t1p")
                    for pp in range(4):
                        base = (grp * 4 + pp) * 120
                        nc.tensor.transpose(
                            pa[0:120, pp * 128:pp * 128 + 128],
                            t1g_flat[:, base:base + 120],
                            idn_sb[:],
                        )
                    evac_copy(t1p[:], pa[0:120, 0:512])
                    for pp in range(4):
                        nc.tensor.matmul(
                            pm[:, pp * 128:pp * 128 + 120],
                            t1p[:, pp * 128:(pp + 1) * 128],
                            q6_sb[:, 0:120],
                            start=True,
                            stop=True,
                        )
                    # pm [g; (pp, ijf, h6)] -> t3g[g; ijf, 24h-slice]
                    evac_copy(
                        t3g[:, :, grp * 24:(grp + 1) * 24].rearrange(
                            "p i (pp h) -> p pp i h", pp=4
                        ),
                        pm[:].rearrange("p (pp x) -> p pp x", pp=4)[:, :, 0:120]
                        .rearrange("p pp (i h) -> p pp i h", h=6),
                    )
                # tail: one 6-pack (h 120..125) + one 2-pack (h 126..127)
                pa32 = ps_mid.tile([128, 512], F32, tag="mid")
                pa = pa32[:].bitcast(FP16)
                pm = ps_mid.tile([128, 512], F32, tag="mid")
                t1p = t1p_pool.tile([120, 512], FP16, tag="t1p")
                nc.tensor.transpose(
                    pa[0:120, 0:128], t1g_flat[:, 2400:2520], idn_sb[:]
                )
                nc.tensor.transpose(
                    pa[0:40, 128:256], t1g_flat[:, 2520:2560], idn_sb[:]
                )
                evac_copy(t1p[:, 0:256], pa[0:120, 0:256])
                nc.tensor.matmul(
                    pm[:, 0:120], t1p[:, 0:128], q6_sb[:, 0:120],
                    start=True, stop=True,
                )
                nc.tensor.matmul(
                    pm[:, 128:168], t1p[0:40, 128:256], q2_sb[:, 0:40],
                    start=True, stop=True,
                )
                evac_copy(
                    t3g[:, :, 120:126],
                    pm[:, 0:120].rearrange("p (i h) -> p i h", h=6),
                )
                evac_copy(
                    t3g[:, :, 126:128],
                    pm[:, 128:168].rearrange("p (i h) -> p i h", h=2),
                )
                t3gs.append(t3g)

            def load_rblk(gblk):
                rblk = rblk_pool.tile([128, 5, 1024], FP16, tag="rblk")
                nc.sync.dma_start(rblk[:], rt.ap()[gblk])
                rblks.append(rblk)

            def make_s4(qq, t3gs_, rblks_):
                def emit_ij(ij):
                    for kh in range(2):
                        ps4 = ps_s4.tile([128, 512], F32, tag="s4")  # 1 bank
                        for half in range(2):
                            for f in range(5):
                                lhsT = t3gs_[half][:, ij * 5 + f, :]
                                nc.tensor.matmul(
                                    ps4[:],
                                    lhsT,
                                    rblks_[half][:, f, kh * 512:(kh + 1) * 512],
                                    start=(half == 0 and f == 0),
                                    stop=(half == 1 and f == 4),
                                )
                        dst = res_sb[:, ij * 1024 + kh * 512:ij * 1024 + (kh + 1) * 512]
                        if qq == 0:
                            evac_copy(dst, ps4[:])
                        else:
                            nc.vector.tensor_add(dst, dst, ps4[:])
                    if qq == 3:
                        nc.sync.dma_start(
                            res.ap()[:, ij * 1024:(ij + 1) * 1024],
                            res_sb[:, ij * 1024:(ij + 1) * 1024],
                        )

                return emit_ij

            # software pipeline: flip/mix trails step1 by one gblk
            prev_t1g = None
            for gblk in range(8):
                t1g = step1(gblk)
                load_rblk(gblk)
                if prev_t1g is not None:
                    flip_mix(prev_t1g)
                    if gblk % 2 == 0:  # finished t3g of gblk-1 (odd): pair done
                        q = (gblk - 1) // 2
                        pending_s4_emitters.append(
                            (make_s4(q, t3gs[-2:], rblks[gblk - 2:gblk]), 4)
                        )
                prev_t1g = t1g
            evac_scalar_bias[0] = True
            flip_mix(prev_t1g)  # gblk 7
            pending_s4_emitters.append((make_s4(3, t3gs[-2:], rblks[6:8]), 4))

            # flush remaining deferred step-4 work
            for emit, n in pending_s4_emitters:
                for ij in range(n):
                    emit(ij)
            pending_s4_emitters.clear()
    nc.compile()
    return nc


def _host_inputs(psi_flat, L, M1, M2, R):
    # psi[a,ce,g]: a=ac*128+a_lo, g=gblk*128+g_lo -> [gblk, a_lo, ce, ac, g_lo]
    psi = np.ascontiguousarray(
        psi_flat.reshape(8, 128, 4, 8, 128).transpose(3, 1, 2, 0, 4)
    ).astype(np.float16)
    # R[f,k,g] -> [gblk, g_lo, f, k]
    RT = np.ascontiguousarray(
        R.transpose(2, 0, 1).reshape(8, 128, 5, 1024)
    ).astype(np.float16)
    # Q rows permuted to (ce, b) order to match t1g layout [h, ce, b]
    Q = np.einsum("bdic,dfje->cebijf", M1, M2).reshape(20, 20).astype(np.float32)
    rows = np.arange(20)
    Q6P = np.zeros((120, 128), np.float32)
    for g6 in range(6):
        Q6P[np.ix_(g6 * 20 + rows, rows * 6 + g6)] = Q
    Q2P = np.zeros((40, 128), np.float32)
    for g2 in range(2):
        Q2P[np.ix_(g2 * 20 + rows, rows * 2 + g2)] = Q
    idn = np.eye(128, dtype=np.float16)
    q6h = Q6P.astype(np.float16)
    q2h = Q2P.astype(np.float16)
    in_maps = []
    for c in range(NCORES):
        # L[b, h_shard, a] -> lt[a_lo, ac, h, b]
        LT = np.ascontiguousarray(
            L[:, c * H:(c + 1) * H, :].reshape(5, H, 8, 128).transpose(3, 2, 1, 0)
        ).astype(np.float16)
        in_maps.append({"psi": psi, "lt": LT, "rt": RT, "q6": q6h, "q2": q2h, "idn": idn})
    return in_maps


def kernel(**inputs):
    psi_flat = np.asarray(inputs["psi_flat"], np.float32)
    L = np.asarray(inputs["L"], np.float32)
    M1 = np.asarray(inputs["M1"], np.float32)
    M2 = np.asarray(inputs["M2"], np.float32)
    R = np.asarray(inputs["R"], np.float32)

    global _nc_cache
    if _nc_cache is None:
        _nc_cache = _build_nc()
    nc = _nc_cache

    in_maps = _host_inputs(psi_flat, L, M1, M2, R)
    out = bass_utils.run_bass_kernel_spmd(nc, in_maps, core_ids=list(range(NCORES)))
    parts = [out.results[c]["res"] for c in range(NCORES)]
    return np.concatenate(parts, axis=0).reshape(-1)


# revision 29
# speedup vs baseline: 1.0810x; 1.0211x over previous
"""DMRG two-site effective Hamiltonian application (ApplyMPO) on 8 trn2 cores.

Math (reference):
  res[h,i,j,k] = sum_{a,b,c,d,e,f,g} L[b,h,a] M1[b,d,i,c] M2[d,f,j,e]
                                     R[f,k,g] psi[a,c,e,g]

Device algorithm (per core, output bond h sharded 8 x 128), all fp16:
  Q[(c,e,b),(i,j,f)] = sum_d M1[b,d,i,c] M2[d,f,j,e]            (host, 400 els)
  step1: T1g[g; h,ce,b] = sum_a psi[a,ce,g] L[b,h,a]   (PE, K=a, g-major out)
  flipA: T1P[(h6,ce,b); g] = transpose of T1g 6-h-column packs  (PE transpose)
  mix:   T3G[g; (ijf,h6)] = T1P^T @ Q6P  (Q6P = block-diag Q over h6)
  step4: res[h; (i,j),k] += T3G[:,ijf,:]^T @ R^T[f][g,k]        (PE, K=g, acc f)
Single transpose stage (mix lands g-major for step4); PE stream is software
pipelined: flip/mix of gblk-1 is emitted after step1 of gblk, so evacuation
latency never blocks ready matmuls.
"""

import numpy as np

import concourse.bacc as bacc
import concourse.mybir as mybir
import concourse.tile as tile
from concourse import bass_utils

F32 = mybir.dt.float32
FP16 = mybir.dt.float16
BF16 = mybir.dt.bfloat16

CHI = 1024
W = 5
D = 2
NCORES = 8
H = CHI // NCORES  # 128, h rows per core

_nc_cache = None


def _build_nc():
    nc = bacc.Bacc("TRN2", target_bir_lowering=False)
    # host-prearranged:
    #   psi[gblk, a_lo, ce, ac, g_lo]  (lhsT tiles for step1: [a; g] per (ce,ac))
    #   lt[a_lo, ac, h, b]             (rhs for step1: [a; (h,b)] per ac)
    #   rt[gblk, g_lo, f, k]           (rhs for step4)
    psi = nc.dram_tensor("psi", [8, 128, 4, 8, 128], FP16, kind="ExternalInput")
    lt = nc.dram_tensor("lt", [128, 8, H, 5], FP16, kind="ExternalInput")
    rt = nc.dram_tensor("rt", [8, 128, 5, 1024], FP16, kind="ExternalInput")
    q6 = nc.dram_tensor("q6", [120, 128], FP16, kind="ExternalInput")
    q2 = nc.dram_tensor("q2", [40, 128], FP16, kind="ExternalInput")
    idn = nc.dram_tensor("idn", [128, 128], FP16, kind="ExternalInput")
    res = nc.dram_tensor("res", [H, 4096], BF16, kind="ExternalOutput")  # h;(i,j,k)

    with tile.TileContext(nc) as tc:
        with (
            tc.tile_pool(name="const", bufs=1) as const_pool,
            tc.tile_pool(name="psis", bufs=3) as psi_pool,
            tc.tile_pool(name="t1", bufs=3) as t1_pool,
            tc.tile_pool(name="t1p", bufs=3) as t1p_pool,
            tc.tile_pool(name="t3g", bufs=5) as t3g_pool,
            tc.tile_pool(name="rblk", bufs=5) as rblk_pool,
            tc.tile_pool(name="resp", bufs=1) as res_pool,
            tc.tile_pool(name="ps_s1a", bufs=2, space="PSUM") as ps_s1a,
            tc.tile_pool(name="ps_mid", bufs=3, space="PSUM") as ps_mid,
            tc.tile_pool(name="ps_s4", bufs=3, space="PSUM") as ps_s4,
        ):
            # ---- static loads ----
            lt_sb = const_pool.tile([128, 8, H * 5], FP16)  # [a_lo; ac, (h,b)]
            lt_r = lt.ap().rearrange("p ac h b -> p ac (h b)")
            nc.sync.dma_start(lt_sb[:, 0], lt_r[:, 0])
            q6_sb = const_pool.tile([120, 128], FP16)
            q2_sb = const_pool.tile([40, 128], FP16)
            idn_sb = const_pool.tile([128, 128], FP16)
            res_sb = res_pool.tile([128, 4096], BF16)

            evac_ct = 0
            evac_scalar_bias = [False]

            def evac_copy(out, in_):
                # only DVE and ACT can read PSUM; alternate them 1:1.
                # Under scalar bias (endgame), give ACT 2 of 3 so the DVE
                # queue stays clear for the step-4 res adds.
                nonlocal evac_ct
                evac_ct += 1
                use_scalar = (
                    evac_ct % 3 != 0 if evac_scalar_bias[0] else evac_ct % 2 == 0
                )
                if use_scalar:
                    nc.scalar.copy(out, in_)
                else:
                    nc.vector.tensor_copy(out, in_)

            pending_s4_emitters = []
            t3gs = []
            rblks = []

            # PE pstate warmup: junk matmuls while startup DMAs are in flight
            junk = const_pool.tile([128, 512], FP16)
            nc.vector.memset(junk[:], 0.001)
            wps = ps_s4.tile([128, 512], F32, tag="s4")
            for _ in range(32):
                nc.tensor.matmul(wps[:], junk[:, 0:128], junk[:], start=True, stop=True)

            def step1(gblk):
                psig = psi_pool.tile([128, 4, 8, 128], FP16, tag="psi")
                if gblk == 0:
                    # per-ce psi slices interleaved with lt chunks so the first
                    # step1 chain starts as early as possible
                    nc.sync.dma_start(psig[:, 0, 0:4], psi.ap()[gblk, :, 0, 0:4])
                    nc.sync.dma_start(psig[:, 0, 4:8], psi.ap()[gblk, :, 0, 4:8])
                    for ac in range(1, 4):
                        nc.sync.dma_start(lt_sb[:, ac], lt_r[:, ac])
                    nc.sync.dma_start(psig[:, 1], psi.ap()[gblk, :, 1])
                    for ac in range(4, 8):
                        nc.sync.dma_start(lt_sb[:, ac], lt_r[:, ac])
                    nc.sync.dma_start(psig[:, 2], psi.ap()[gblk, :, 2])
                    nc.sync.dma_start(q6_sb[:], q6.ap())
                    nc.sync.dma_start(psig[:, 3], psi.ap()[gblk, :, 3])
                    nc.sync.dma_start(q2_sb[:], q2.ap())
                    nc.sync.dma_start(idn_sb[:], idn.ap())
                else:
                    nc.sync.dma_start(psig[:], psi.ap()[gblk])
                t1g = t1_pool.tile([128, 128, 4, 5], FP16, tag="t1g")  # g;h,ce,b
                for ce in range(4):
                    p1a = ps_s1a.tile([128, 510], F32, tag="s1a")
                    p1b32 = ps_mid.tile([128, 512], F32, tag="mid")
                    p1b = p1b32[:, 0:130]
                    for ac in range(8):
                        lhsT = psig[:, ce, ac]  # [a; g]
                        nc.tensor.matmul(
                            p1a[:],
                            lhsT,
                            lt_sb[:, ac, 0:510],
                            start=(ac == 0),
                            stop=(ac == 7),
                        )
                        nc.tensor.matmul(
                            p1b,
                            lhsT,
                            lt_sb[:, ac, 510:640],
                            start=(ac == 0),
                            stop=(ac == 7),
                        )
                    # psum [g; (h,b)] -> t1g[g; h, ce, b]: runs of 5 both sides
                    evac_copy(
                        t1g[:, 0:102, ce, :],
                        p1a[:].rearrange("p (h b) -> p h b", b=5),
                    )
                    evac_copy(
                        t1g[:, 102:128, ce, :],
                        p1b.rearrange("p (h b) -> p h b", b=5),
                    )
                    if pending_s4_emitters:
                        pending_s4_emitters[0][0](ce)
                        if ce == 3:
                            pending_s4_emitters.pop(0)
                return t1g

            def flip_mix(t1g):
                # T3G[g; ijf, h] from T1g, one transpose stage + block-diag Q
                t3g = t3g_pool.tile([128, 20, 128], FP16, tag="t3g")
                t1g_flat = t1g[:].rearrange("p h ce b -> p (h ce b)")
                # 5 groups of 4 six-packs (h 0..119), then (6,2) tail (h 120..127)
                for grp in range(5):
                    pa32 = ps_mid.tile([128, 512], F32, tag="mid")
                    pa = pa32[:].bitcast(FP16)  # [128, 1024] fp16 view
                    pm = ps_mid.tile([128, 512], F32, tag="mid")
                    t1p = t1p_pool.tile([120, 512], FP16, tag="t1p")
                    for pp in range(4):
                        base = (grp * 4 + pp) * 120
                        nc.tensor.transpose(
                            pa[0:120, pp * 128:pp * 128 + 128],
                            t1g_flat[:, base:base + 120],
                            idn_sb[:],
                        )
                    evac_copy(t1p[:], pa[0:120, 0:512])
                    for pp in range(4):
                        nc.tensor.matmul(
                            pm[:, pp * 128:pp * 128 + 120],
                            t1p[:, pp * 128:(pp + 1) * 128],
                            q6_sb[:, 0:120],
                            start=True,
                            stop=True,
                        )
                    # pm [g; (pp, ijf, h6)] -> t3g[g; ijf, 24h-slice]
                    evac_copy(
                        t3g[:, :, grp * 24:(grp + 1) * 24].rearrange(
                            "p i (pp h) -> p pp i h", pp=4
                        ),
                        pm[:].rearrange("p (pp x) -> p pp x", pp=4)[:, :, 0:120]
                        .rearrange("p pp (i h) -> p pp i h", h=6),
                    )
                # tail: one 6-pack (h 120..125) + one 2-pack (h 126..127)
                pa32 = ps_mid.tile([128, 512], F32, tag="mid")
                pa = pa32[:].bitcast(FP16)
                pm = ps_mid.tile([128, 512], F32, tag="mid")
                t1p = t1p_pool.tile([120, 512], FP16, tag="t1p")
                nc.tensor.transpose(
                    pa[0:120, 0:128], t1g_flat[:, 2400:2520], idn_sb[:]
                )
                nc.tensor.transpose(
                    pa[0:40, 128:256], t1g_flat[:, 2520:2560], idn_sb[:]
                )
                evac_copy(t1p[:, 0:256], pa[0:120, 0:256])
                nc.tensor.matmul(
                    pm[:, 0:120], t1p[:, 0:128], q6_sb[:, 0:120],
                    start=True, stop=True,
                )
                nc.tensor.matmul(
                    pm[:, 128:168], t1p[0:40, 128:256], q2_sb[:, 0:40],
                    start=True, stop=True,
                )
                evac_copy(
                    t3g[:, :, 120:126],
                    pm[:, 0:120].rearrange("p (i h) -> p i h", h=6),
                )
                evac_copy(
                    t3g[:, :, 126:128],
                    pm[:, 128:168].rearrange("p (i h) -> p i h", h=2),
                )
                t3gs.append(t3g)

            def load_rblk(gblk):
                rblk = rblk_pool.tile([128, 5, 1024], FP16, tag="rblk")
                nc.sync.dma_start(rblk[:], rt.ap()[gblk])
                rblks.append(rblk)

            def make_s4(qq, t3gs_, rblks_):
                def emit_ij(ij):
                    for kh in range(2):
                        ps4 = ps_s4.tile([128, 512], F32, tag="s4")  # 1 bank
                        for half in range(2):
                            for f in range(5):
                                lhsT = t3gs_[half][:, ij * 5 + f, :]
                                nc.tensor.matmul(
                                    ps4[:],
                                    lhsT,
                                    rblks_[half][:, f, kh * 512:(kh + 1) * 512],
                                    start=(half == 0 and f == 0),
                                    stop=(half == 1 and f == 4),
                                )
                        dst = res_sb[:, ij * 1024 + kh * 512:ij * 1024 + (kh + 1) * 512]
                        if qq == 0:
                            evac_copy(dst, ps4[:])
                        else:
                            nc.vector.tensor_add(dst, dst, ps4[:])
                    if qq == 3:
                        nc.sync.dma_start(
                            res.ap()[:, ij * 1024:(ij + 1) * 1024],
                            res_sb[:, ij * 1024:(ij + 1) * 1024],
                        )

                return emit_ij

            # software pipeline: flip/mix trails step1 by one gblk
            prev_t1g = None
            for gblk in range(8):
                t1g = step1(gblk)
                load_rblk(gblk)
                if prev_t1g is not None:
                    flip_mix(prev_t1g)
                    if gblk % 2 == 0:  # finished t3g of gblk-1 (odd): pair done
                        q = (gblk - 1) // 2
                        pending_s4_emitters.append(
                            (make_s4(q, t3gs[-2:], rblks[gblk - 2:gblk]), 4)
                        )
                prev_t1g = t1g
            evac_scalar_bias[0] = True
            flip_mix(prev_t1g)  # gblk 7
            pending_s4_emitters.append((make_s4(3, t3gs[-2:], rblks[6:8]), 4))

            # flush remaining deferred step-4 work
            for emit, n in pending_s4_emitters:
                for ij in range(n):
                    emit(ij)
            pending_s4_emitters.clear()
    nc.compile()
    return nc


def _host_inputs(psi_flat, L, M1, M2, R):
    # psi[a,ce,g]: a=ac*128+a_lo, g=gblk*128+g_lo -> [gblk, a_lo, ce, ac, g_lo]
    psi = np.ascontiguousarray(
        psi_flat.reshape(8, 128, 4, 8, 128).transpose(3, 1, 2, 0, 4)
    ).astype(np.float16)
    # R[f,k,g] -> [gblk, g_lo, f, k]
    RT = np.ascontiguousarray(
        R.transpose(2, 0, 1).reshape(8, 128, 5, 1024)
    ).astype(np.float16)
    # Q rows permuted to (ce, b) order to match t1g layout [h, ce, b]
    Q = np.einsum("bdic,dfje->cebijf", M1, M2).reshape(20, 20).astype(np.float32)
    rows = np.arange(20)
    Q6P = np.zeros((120, 128), np.float32)
    for g6 in range(6):
        Q6P[np.ix_(g6 * 20 + rows, rows * 6 + g6)] = Q
    Q2P = np.zeros((40, 128), np.float32)
    for g2 in range(2):
        Q2P[np.ix_(g2 * 20 + rows, rows * 2 + g2)] = Q
    idn = np.eye(128, dtype=np.float16)
    q6h = Q6P.astype(np.float16)
    q2h = Q2P.astype(np.float16)
    in_maps = []
    for c in range(NCORES):
        # L[b, h_shard, a] -> lt[a_lo, ac, h, b]
        LT = np.ascontiguousarray(
            L[:, c * H:(c + 1) * H, :].reshape(5, H, 8, 128).transpose(3, 2, 1, 0)
        ).astype(np.float16)
        in_maps.append({"psi": psi, "lt": LT, "rt": RT, "q6": q6h, "q2": q2h, "idn": idn})
    return in_maps


def kernel(**inputs):
    psi_flat = np.asarray(inputs["psi_flat"], np.float32)
    L = np.asarray(inputs["L"], np.float32)
    M1 = np.asarray(inputs["M1"], np.float32)
    M2 = np.asarray(inputs["M2"], np.float32)
    R = np.asarray(inputs["R"], np.float32)

    global _nc_cache
    if _nc_cache is None:
        _nc_cache = _build_nc()
    nc = _nc_cache

    in_maps = _host_inputs(psi_flat, L, M1, M2, R)
    out = bass_utils.run_bass_kernel_spmd(nc, in_maps, core_ids=list(range(NCORES)))
    parts = [np.asarray(out.results[c]["res"]) for c in range(NCORES)]
    return np.concatenate(parts, axis=0).astype(np.float32).reshape(-1)
